# revision 14
# baseline (speedup 1.0000x reference)
"""Local 3x3 attention (kernel_size=3) as a raw-Bass kernel on 8 TRN2
NeuronCores, data-parallel over the batch dim (4 batches per core).

Math per batch (d=128, H=W=64):
  logits[kk,h,w] = sum_d q[d,h,w] * k[d,h+di,w+dj] * scale,  kk=(di,dj) in 3x3
  attn = softmax over the 9 neighbors (zero-padded: OOB neighbors contribute
         exp(0)=1 to the denominator, 0 to the numerator)
  out[h,w,d] = sum_kk attn[kk,h,w] * v[d,h+di,w+dj]

Per image-row-pair g = 32b + t (rows 2t, 2t+1; 128 partitions = (row, w)):
  L = K_pair^T @ Q_4rows          PE: [128d,128] x [128d,256] -> PSUM [128,256]
  E = exp(scale*L)                ACT, PSUM->SBUF bf16
  T = E * bandmask                DVE bf16 (zeroes |dw|>1; junk blocks unread)
  U = sum_di T_blk^T @ [V^T | 1]  PE: 3 accumulating matmuls; the ones column
                                  of V^T yields the softmax denominator
  out = U[:,:128] / (U[:,128]+n_phantom)   DVE add+recip, ACT scaled copy
V^T tiles come from the DMA xbar transpose (bf16, SBUF->SBUF). q,k,v are cast
fp32->bf16 on GPSIMD. Raw Bass with explicit standalone semaphore waits:
Tile's embedded-wait encoding limits (TensorTensor holds 1 sync wait,
Activation 2) make its schedule uncompilable for this dependency pattern.
"""

import numpy as np
import ml_dtypes

B, D, H, W = 32, 128, 64, 64
N_CORES = 8
B_LOC = B // N_CORES
PAIRS = H // 2
NPAIR = B_LOC * PAIRS  # 128 pair-iterations per core
SCALE = float(D) ** -0.5

_cache = {}


def _mask_np():
    # [128, 256] bf16: partition p=(ri, wp), free f=(hj, wq).
    # k-row r = r0 + ri; q-row h = r0 - 1 + hj. Valid iff |h - r| <= 1 and
    # |wp - wq| <= 1. Blocks with |h - r| > 1 are never read downstream.
    m = np.zeros((128, 256), np.float32)
    for ri in range(2):
        for hj in range(4):
            if abs(hj - 1 - ri) <= 1:
                for wp in range(W):
                    lo, hi = max(0, wp - 1), min(W, wp + 2)
                    m[ri * W + wp, hj * W + lo:hj * W + hi] = 1.0
    return m.astype(ml_dtypes.bfloat16)


def _noob_np():
    # [128, 32] fp32: phantom (out-of-bounds) neighbor count per output pixel.
    # Column t holds rows (2t, 2t+1); partition p=(i, w) -> h = 2t + i//W.
    cnt = np.zeros((H, W), np.float32)
    for h in range(H):
        for w in range(W):
            n = 0
            for di in (-1, 0, 1):
                for dj in (-1, 0, 1):
                    if not (0 <= h + di < H and 0 <= w + dj < W):
                        n += 1
            cnt[h, w] = n
    out = np.zeros((128, PAIRS), np.float32)
    for t in range(PAIRS):
        out[:W, t] = cnt[2 * t]
        out[W:, t] = cnt[2 * t + 1]
    return out


def _nmm(s):
    # number of AV matmuls for pair s (edge rows lack one neighbor)
    ss = s % PAIRS
    return 1 + (1 if ss > 0 else 0) + (1 if ss < PAIRS - 1 else 0)


def _build_nc(nrep=1):
    import concourse.bass as bass
    import concourse.mybir as mybir
    from contextlib import ExitStack

    f32 = mybir.dt.float32
    bf16 = mybir.dt.bfloat16
    AF = mybir.ActivationFunctionType
    OP = mybir.AluOpType

    NB = nrep * B_LOC          # logical batches (timing: nrep passes)
    NPT = nrep * NPAIR         # total pair iterations
    nc = bass.Bass()
    q = nc.declare_dram_parameter("q", [B_LOC, D, H, W], f32, False)
    k = nc.declare_dram_parameter("k", [B_LOC, D, H, W], f32, False)
    v = nc.declare_dram_parameter("v", [B_LOC, D, H, W], f32, False)
    mask = nc.declare_dram_parameter("mask", [128, 256], bf16, False)
    noob = nc.declare_dram_parameter("noob", [128, PAIRS], f32, False)
    out = nc.declare_dram_parameter("out", [B_LOC, H, W, D], f32, True)

    NQF = 2    # fp32 staging slots per input tensor
    NBF = 2    # bf16 batch slots
    NL = 4     # L PSUM banks
    NU = 4     # U PSUM banks
    NE = 4     # E slots
    NT = 8     # T slots
    NVT = 8    # V^T slots
    NO = 4     # output staging slots

    ctx = ExitStack()
    with ctx:
        mask_sb = ctx.enter_context(nc.sbuf_tensor("mask_sb", [128, 256], bf16))
        noob_sb = ctx.enter_context(nc.sbuf_tensor("noob_sb", [128, PAIRS], f32))
        qf = [ctx.enter_context(nc.sbuf_tensor(f"qf{i}", [128, H, W], f32))
              for i in range(NQF)]
        kf = [ctx.enter_context(nc.sbuf_tensor(f"kf{i}", [128, H, W], f32))
              for i in range(NQF)]
        vf = [ctx.enter_context(nc.sbuf_tensor(f"vf{i}", [128, H, W], f32))
              for i in range(NQF)]
        Qb = [ctx.enter_context(nc.sbuf_tensor(f"Qb{i}", [128, H + 2, W], bf16))
              for i in range(NBF)]
        Kb = [ctx.enter_context(nc.sbuf_tensor(f"Kb{i}", [128, H, W], bf16))
              for i in range(NBF)]
        Vb = [ctx.enter_context(nc.sbuf_tensor(f"Vb{i}", [128, H, W], bf16))
              for i in range(NBF)]
        E = [ctx.enter_context(nc.sbuf_tensor(f"E{i}", [128, 256], bf16))
             for i in range(NE)]
        T = [ctx.enter_context(nc.sbuf_tensor(f"T{i}", [128, 256], bf16))
             for i in range(NT)]
        VT = [ctx.enter_context(nc.sbuf_tensor(f"VT{i}", [128, 129], bf16))
              for i in range(NVT)]
        O = [ctx.enter_context(nc.sbuf_tensor(f"O{i}", [128, 128], f32))
             for i in range(NO)]
        dent = [ctx.enter_context(nc.sbuf_tensor(f"dent{i}", [128, 1], f32))
                for i in range(NO)]
        rd = [ctx.enter_context(nc.sbuf_tensor(f"rd{i}", [128, 1], f32))
              for i in range(NO)]
        L = [ctx.enter_context(nc.psum_tensor(f"L{i}", [128, 512], f32))
             for i in range(NL)]
        U = [ctx.enter_context(nc.psum_tensor(f"U{i}", [128, 512], f32))
             for i in range(NU)]

        sem_pe = ctx.enter_context(nc.semaphore("s_pe"))
        sem_act = ctx.enter_context(nc.semaphore("s_act"))
        sem_dve = ctx.enter_context(nc.semaphore("s_dve"))
        sem_gp = ctx.enter_context(nc.semaphore("s_gp"))
        dq_q = ctx.enter_context(nc.semaphore("dq_q"))
        dq_k = ctx.enter_context(nc.semaphore("dq_k"))
        dq_v = ctx.enter_context(nc.semaphore("dq_v"))
        dq_c = ctx.enter_context(nc.semaphore("dq_c"))
        vts = [ctx.enter_context(nc.semaphore(f"d_vt{i}")) for i in range(NVT)]
        osem = [ctx.enter_context(nc.semaphore(f"d_o{i}")) for i in range(NO)]

        # ---- Phase A: walk the schedule, record completion counter values
        ev = {}

        def plan():
            gp = 2 * NBF                       # Qb pad memsets
            for b in range(NB):
                for nm in ("q", "k", "v"):
                    ev[f"ld_{nm}{b}"] = 16 * (b + 1)
                for nm in ("q", "k", "v"):
                    gp += 1
                    ev[f"cast_{nm}{b}"] = gp
            pe = 0
            for g in range(NPT):
                pe += 1
                ev[f"qk{g}"] = pe
                for s in ([g - 1] if g % PAIRS >= 1 else []) + \
                         ([g] if g % PAIRS == PAIRS - 1 else []):
                    pe += _nmm(s)
                    ev[f"av{s}"] = pe
            act = 0
            for g in range(NPT):
                act += 1
                ev[f"exp{g}"] = act
                for s in ([g - 1] if g % PAIRS >= 1 else []) + \
                         ([g] if g % PAIRS == PAIRS - 1 else []):
                    act += 1
                    ev[f"norm{s}"] = act
            dve = NVT                          # VT ones-column memsets
            for g in range(NPT):
                dve += 1
                ev[f"mask{g}"] = dve
                for s in ([g - 1] if g % PAIRS >= 1 else []) + \
                         ([g] if g % PAIRS == PAIRS - 1 else []):
                    dve += 2
                    ev[f"recip{s}"] = dve
            for g in range(NPT):
                ev[f"vt{g}"] = 16 * (g // NVT + 1)   # per-slot sem value
            for s in range(NPT):
                ev[f"st{s}"] = 16 * (s // NO + 1)    # per-slot sem value

        plan()

        # ---- Phase B: emit per-engine streams with standalone waits
        hw = {}

        def wge(eng, en, sem, val):
            if val is None or val <= 0:
                return
            key = (en, id(sem))
            if hw.get(key, 0) >= val:
                return
            hw[key] = val
            eng.wait_ge(sem, val)

        def pend(g):
            # pairs s whose tail work runs at loop iteration g
            return ([g - 1] if g % PAIRS >= 1 else []) + \
                   ([g] if g % PAIRS == PAIRS - 1 else [])

        with nc.Block() as block:

            @block.gpsimd
            def _(g_e):
                for i in range(NBF):
                    g_e.memset(Qb[i][:, 0, :], 0.0).then_inc(sem_gp, 1)
                    g_e.memset(Qb[i][:, H + 1, :], 0.0).then_inc(sem_gp, 1)
                for b in range(NB):
                    wge(g_e, "gp", dq_q, ev[f"ld_q{b}"])
                    if b >= NBF:
                        # av(b-2,31) transitively covers QK reads of Qb/Kb
                        # and (via its vt waits) all b-2 transposes of Vb.
                        wge(g_e, "gp", sem_pe, ev[f"av{32 * (b - NBF) + 31}"])
                    g_e.tensor_copy(out=Qb[b % NBF][:, 1:H + 1, :],
                                    in_=qf[b % NQF][:]).then_inc(sem_gp, 1)
                    wge(g_e, "gp", dq_k, ev[f"ld_k{b}"])
                    g_e.tensor_copy(out=Kb[b % NBF][:],
                                    in_=kf[b % NQF][:]).then_inc(sem_gp, 1)
                    wge(g_e, "gp", dq_v, ev[f"ld_v{b}"])
                    g_e.tensor_copy(out=Vb[b % NBF][:],
                                    in_=vf[b % NQF][:]).then_inc(sem_gp, 1)

            @block.sync
            def _(s_e):
                s_e.dma_start(out=mask_sb[:, :],
                              in_=mask[:, :]).then_inc(dq_c, 16)
                s_e.dma_start(out=noob_sb[:, :],
                              in_=noob[:, :]).then_inc(dq_c, 16)
                for b in range(NB):
                    if b >= NQF:
                        wge(s_e, "sp", sem_gp, ev[f"cast_v{b - NQF}"])
                    s_e.dma_start(out=qf[b % NQF][:],
                                  in_=q[b % B_LOC]).then_inc(dq_q, 16)
                    s_e.dma_start(out=kf[b % NQF][:],
                                  in_=k[b % B_LOC]).then_inc(dq_k, 16)
                    s_e.dma_start(out=vf[b % NQF][:],
                                  in_=v[b % B_LOC]).then_inc(dq_v, 16)
                    for t in range(PAIRS):
                        g = 32 * b + t
                        wge(s_e, "sp", sem_gp, ev[f"cast_v{b}"])
                        if g - NVT + 1 >= 0:
                            wge(s_e, "sp", sem_pe, ev.get(f"av{g - NVT + 1}"))
                        s_e.dma_start_transpose(
                            VT[g % NVT][:, 0:128],
                            Vb[b % NBF][:, 2 * t:2 * t + 2, :],
                        ).then_inc(vts[g % NVT], 16)
                        for s in pend(g):
                            wge(s_e, "sp", sem_act, ev[f"norm{s}"])
                            ss = s % PAIRS
                            s_e.dma_start(
                                out=out[b % B_LOC,
                                        2 * ss:2 * ss + 2].rearrange(
                                    "h w d -> (h w) d"),
                                in_=O[s % NO][:]).then_inc(osem[s % NO], 16)
                for i in range(NO):
                    s_e.wait_ge(osem[i], 16 * (NPT // NO))

            @block.tensor
            def _(t_e):
                def av(s):
                    b = s // PAIRS
                    ss = s % PAIRS
                    last = PAIRS * b + PAIRS - 1
                    wge(t_e, "pe", sem_dve, ev[f"mask{min(s + 1, last)}"])
                    for sn in ([s - 1] if ss > 0 else []) + [s] + \
                              ([s + 1] if ss < PAIRS - 1 else []):
                        wge(t_e, "pe", vts[sn % NVT], ev[f"vt{sn}"])
                    if s - NU >= 0:
                        wge(t_e, "pe", sem_act, ev.get(f"norm{s - NU}"))
                    t_e.matmul(U[s % NU][:, 0:129], T[s % NT][:, 64:192],
                               VT[s % NVT][:, :], start=True, stop=False,
                               skip_group_check=True).then_inc(sem_pe, 1)
                    if ss > 0:
                        t_e.matmul(U[s % NU][0:64, 0:129],
                                   T[(s - 1) % NT][64:128, 192:256],
                                   VT[(s - 1) % NVT][64:128, :],
                                   start=False, stop=(ss == PAIRS - 1),
                                   skip_group_check=True).then_inc(sem_pe, 1)
                    if ss < PAIRS - 1:
                        t_e.matmul(U[s % NU][64:128, 0:129],
                                   T[(s + 1) % NT][0:64, 0:64],
                                   VT[(s + 1) % NVT][0:64, :],
                                   start=False, stop=True,
                                   skip_group_check=True).then_inc(sem_pe, 1)

                for g in range(NPT):
                    b, t = divmod(g, PAIRS)
                    wge(t_e, "pe", sem_gp, ev[f"cast_k{b}"])
                    if g - NL >= 0:
                        wge(t_e, "pe", sem_act, ev[f"exp{g - NL}"])
                    t_e.matmul(L[g % NL][:, 0:256],
                               Kb[b % NBF][:, 2 * t:2 * t + 2, :],
                               Qb[b % NBF][:, 2 * t:2 * t + 4, :],
                               start=True, stop=True).then_inc(sem_pe, 1)
                    for s in pend(g):
                        av(s)

            @block.scalar
            def _(a_e):
                for g in range(NPT):
                    wge(a_e, "act", sem_pe, ev[f"qk{g}"])
                    if g - NE >= 0:
                        wge(a_e, "act", sem_dve, ev[f"mask{g - NE}"])
                    a_e.activation(E[g % NE][:], L[g % NL][:, 0:256], AF.Exp,
                                   scale=SCALE).then_inc(sem_act, 1)
                    for s in pend(g):
                        wge(a_e, "act", sem_dve, ev[f"recip{s}"])
                        if s - NO >= 0:
                            wge(a_e, "act", osem[s % NO], ev[f"st{s - NO}"])
                        a_e.activation(O[s % NO][:], U[s % NU][:, 0:128],
                                       AF.Copy,
                                       scale=rd[s % NO][:]).then_inc(sem_act, 1)

            @block.vector
            def _(v_e):
                for i in range(NVT):
                    v_e.memset(VT[i][:, 128:129], 1.0).then_inc(sem_dve, 1)
                v_e.wait_ge(dq_c, 32)
                for g in range(NPT):
                    wge(v_e, "dve", sem_act, ev[f"exp{g}"])
                    if g - NT + 1 >= 0:
                        wge(v_e, "dve", sem_pe, ev.get(f"av{g - NT + 1}"))
                    v_e.tensor_tensor(T[g % NT][:], E[g % NE][:], mask_sb[:],
                                      OP.mult).then_inc(sem_dve, 1)
                    for s in pend(g):
                        wge(v_e, "dve", sem_pe, ev[f"av{s}"])
                        if s - NO >= 0:
                            wge(v_e, "dve", sem_act, ev.get(f"norm{s - NO}"))
                            # dent-slot WAR vs our own recip 4 pairs ago
                            wge(v_e, "dve", sem_dve, ev[f"recip{s - NO}"])
                        v_e.tensor_tensor(dent[s % NO][:],
                                          U[s % NU][:, 128:129],
                                          noob_sb[:, (s % PAIRS):
                                                  (s % PAIRS) + 1],
                                          OP.add).then_inc(sem_dve, 1)
                        # same-engine RAW: recip reads dent written just above
                        wge(v_e, "dve", sem_dve, ev[f"recip{s}"] - 1)
                        v_e.reciprocal(rd[s % NO][:],
                                       dent[s % NO][:]).then_inc(sem_dve, 1)

    return nc


def _get_nc():
    if "nc" not in _cache:
        _cache["nc"] = _build_nc()
        _cache["mask"] = np.asarray(_mask_np())
        _cache["noob"] = _noob_np()
    return _cache["nc"]


def kernel(q, k, v):
    from concourse.bass_utils import run_bass_kernel_spmd

    nc = _get_nc()
    q = np.ascontiguousarray(np.asarray(q, np.float32))
    k = np.ascontiguousarray(np.asarray(k, np.float32))
    v = np.ascontiguousarray(np.asarray(v, np.float32))
    in_maps = []
    for c in range(N_CORES):
        s = slice(c * B_LOC, (c + 1) * B_LOC)
        in_maps.append({
            "q": q[s], "k": k[s], "v": v[s],
            "mask": _cache["mask"], "noob": _cache["noob"],
        })
    res = run_bass_kernel_spmd(nc, in_maps, list(range(N_CORES)))
    return np.concatenate([r["out"] for r in res.results], axis=0)
